# revision 9
# baseline (speedup 1.0000x reference)
"""Multi-head attention (softmax(QK^T/sqrt(d))V + scores output) on 8 TRN2 cores.

Sharding: B*H = 64 (batch, head) pairs split contiguously across 8 cores
(8 heads each); every head's full 2048x2048 attention block is computed on
one core with no cross-core communication.

Per-core per-head pipeline:
  A) load Q,K,V [2048,64]; PE-transpose Q,K into QT,KT [64,2048] (d on
     partitions) so both matmul orientations can run off the same operands.
  B) logits^T blocks: PT[st] = (KT_st)^T @ QT via PE (float32r, 1 cyc/row),
     ACT exp -> ET[st] [128,2048] bf16 (unnormalized, feeds PV).
  C) O^T = sum_st V[st]^T ET[st] via PE (bf16), accumulated in PSUM.
  D) logits blocks: P[lt] = (QT_lt)^T @ KT, ACT exp (accum_out gives exact
     fp32 row sums Z), DVE 1/Z + normalize, DMA 1 MiB score tiles out.
  E) PE-transpose O^T back to [l,d] tiles, scale rows by 1/Z, DMA out.
"""

import numpy as np

B, H, L, S, D = 4, 16, 2048, 2048, 64
NCORES = 8
BH = B * H
NH = BH // NCORES  # heads per core
NT = L // 128      # 128-row tiles per head
SCALE = 1.0 / 8.0  # 1/sqrt(64)

_PATCHED = False
_PROG_CACHE = {}


def _apply_tile_patch():
    """walrus CoreV3 codegen rejects a Drain carrying more than a couple of
    sem waits ("Too many sync wait commands"). Split the TileContext exit
    drain's clock-waits across a chain of SP drains, one wait each."""
    global _PATCHED
    if _PATCHED:
        return
    import concourse.tile as tile
    import concourse.mybir as mybir
    from concourse.vector_clock import ScopedClock

    def _patched_drain_and_barrier(self, tick_clock, wait_clock):
        nc = self.nc
        drain_inst = nc.sync.drain()
        wait_clock.add_sem_waits(
            drain_inst.ins, ScopedClock({None: tick_clock.global_clock})
        )
        si = drain_inst.ins.sync_info
        waits = list(si.on_wait) if si is not None and si.on_wait else []
        if len(waits) > 1:
            upd = list(si.on_update) if si is not None and si.on_update else []
            drain_inst.ins.sync_info = mybir.SyncInfo(on_wait=waits[:1], on_update=upd)
            for w in waits[1:]:
                d2 = nc.sync.drain()
                d2.ins.sync_info = mybir.SyncInfo(on_wait=[w], on_update=[])
        nc.all_engine_barrier()
        assert self.sems is not None
        popped = nc._tile_sem_poison_stack.pop()
        assert popped is self._sem_poison
        nc.clear_and_free_semaphores(list(self.sems.allocated().values()))
        nc.all_engine_barrier()

    tile.TileContext._drain_and_barrier = _patched_drain_and_barrier
    _PATCHED = True


def _split_multi_waits(nc):
    """This walrus build rejects any instruction carrying more than one sem
    wait ("Too many sync wait commands"). Hoist extra waits onto preceding
    same-engine NoOps, one wait each."""
    import concourse.mybir as mybir

    for fn in nc.m.functions:
        for blk in fn.blocks:
            if not isinstance(blk, mybir.BasicBlock):
                continue
            out = []
            changed = False
            for inst in blk.instructions:
                si = inst.sync_info
                waits = list(si.on_wait) if si is not None and si.on_wait else []
                if len(waits) > 1:
                    changed = True
                    for i, w in enumerate(waits[:-1]):
                        nop = mybir.InstNoOp(
                            name=f"{inst.name}-sw{i}", ins=[], outs=[])
                        nop.engine = inst.engine
                        nop.sync_info = mybir.SyncInfo(on_wait=[w], on_update=[])
                        out.append(nop)
                    upd = list(si.on_update) if si.on_update else []
                    inst.sync_info = mybir.SyncInfo(
                        on_wait=[waits[-1]], on_update=upd)
                out.append(inst)
            if changed:
                blk.instructions = out


def _build_program(reps=1):
    import concourse.bass as bass
    import concourse.tile as tile
    from concourse import mybir
    from concourse.masks import make_identity

    _apply_tile_patch()
    f32 = mybir.dt.float32
    f16 = mybir.dt.float16
    Exp = mybir.ActivationFunctionType.Exp

    nc = bass.Bass("TRN2", target_bir_lowering=False, debug=False,
                   num_devices=NCORES)
    q_d = nc.dram_tensor("q", [NH, L, D], f32, kind="ExternalInput").ap()
    k_d = nc.dram_tensor("k", [NH, L, D], f32, kind="ExternalInput").ap()
    v_d = nc.dram_tensor("v", [NH, L, D], f32, kind="ExternalInput").ap()
    scores_d = nc.dram_tensor("scores", [NH, L, S], f32, kind="ExternalOutput").ap()
    out_d = nc.dram_tensor("out", [NH, L, D], f32, kind="ExternalOutput").ap()

    with tile.TileContext(nc) as tc:
        from contextlib import ExitStack
        with ExitStack() as ctx:
            cpool = ctx.enter_context(tc.tile_pool(name="const", bufs=1))
            stage = ctx.enter_context(tc.tile_pool(name="stage", bufs=4))
            qkt = ctx.enter_context(tc.tile_pool(name="qkt", bufs=2))
            vpool = ctx.enter_context(tc.tile_pool(name="vp", bufs=2))
            etp = ctx.enter_context(tc.tile_pool(name="et", bufs=17))
            pnp = ctx.enter_context(tc.tile_pool(name="pn", bufs=4))
            otp = ctx.enter_context(tc.tile_pool(name="ot", bufs=2))
            osp = ctx.enter_context(tc.tile_pool(name="os", bufs=2))
            dnp = ctx.enter_context(tc.tile_pool(name="dn", bufs=2))
            zrp = ctx.enter_context(tc.tile_pool(name="zr", bufs=2))
            ppool = ctx.enter_context(tc.tile_pool(name="pp", bufs=2, space="PSUM"))
            smp = ctx.enter_context(tc.tile_pool(name="sm", bufs=2, space="PSUM"))

            ident = cpool.tile([128, 128], f32, tag="ident")
            make_identity(nc, ident[:])

            for _rep in range(reps):
                for h in range(NH):
                    # --- A: loads + Q/K transposes -------------------------
                    sq = stage.tile([128, NT, D], f32, tag="stage")
                    nc.sync.dma_start(sq[:], q_d[h].rearrange("(n p) d -> p n d", p=128))
                    sk = stage.tile([128, NT, D], f32, tag="stage")
                    nc.sync.dma_start(sk[:], k_d[h].rearrange("(n p) d -> p n d", p=128))
                    sv = stage.tile([128, NT, D], f32, tag="stage")
                    nc.sync.dma_start(sv[:], v_d[h].rearrange("(n p) d -> p n d", p=128))

                    qt = qkt.tile([D, L], f16, tag="qt")
                    kt = qkt.tile([D, L], f16, tag="kt")
                    for n in range(NT):
                        pq = smp.tile([128, 512], f32, tag="sm")
                        nc.tensor.transpose(pq[:D, 0:128], sq[:, n, :], ident[:])
                        nc.vector.tensor_copy(qt[:, n * 128:(n + 1) * 128], pq[:D, 0:128])
                        pk = smp.tile([128, 512], f32, tag="sm")
                        nc.tensor.transpose(pk[:D, 0:128], sk[:, n, :], ident[:])
                        nc.vector.tensor_copy(kt[:, n * 128:(n + 1) * 128], pk[:D, 0:128])

                    # V plus a ones column: PV then yields O^T rows 0..63 and
                    # the softmax denominators in row 64.
                    vp = vpool.tile([128, NT, D + 1], f16, tag="vp")
                    nc.vector.tensor_copy(vp[:, :, 0:D], sv[:])
                    nc.vector.memset(vp[:, :, D], 1.0)

                    # --- B: logits^T -> exp -> ET (f16) --------------------
                    et_tiles = []
                    for st in range(NT):
                        et_t = etp.tile([128, S], f16, tag="et")
                        ksl = kt[:, st * 128:(st + 1) * 128]
                        pa = ppool.tile([128, 1536], f32, tag="pp")
                        for c3 in range(3):
                            nc.tensor.matmul(
                                pa[:, c3 * 512:(c3 + 1) * 512], lhsT=ksl,
                                rhs=qt[:, c3 * 512:(c3 + 1) * 512],
                                start=True, stop=True)
                        nc.scalar.activation(et_t[:, 0:1536], pa[:], Exp, scale=SCALE)
                        pb = ppool.tile([128, 1536], f32, tag="pp")
                        nc.tensor.matmul(pb[:, 0:512], lhsT=ksl,
                                         rhs=qt[:, 1536:2048], start=True, stop=True)
                        nc.scalar.activation(et_t[:, 1536:2048], pb[:, 0:512], Exp,
                                             scale=SCALE)
                        et_tiles.append(et_t)

                    # --- C: [O^T; Z] = sum_st V'[st]^T ET[st] --------------
                    ot = otp.tile([D, L], f32, tag="ot")
                    dn = dnp.tile([1, L], f32, tag="dn")
                    for lc in range(4):
                        po = smp.tile([128, 512], f32, tag="sm")
                        for st in range(NT):
                            nc.tensor.matmul(
                                po[:D + 1, :], lhsT=vp[:, st, :],
                                rhs=et_tiles[st][:, lc * 512:(lc + 1) * 512],
                                start=(st == 0), stop=(st == NT - 1))
                        nc.vector.tensor_copy(ot[:, lc * 512:(lc + 1) * 512], po[:D, :])
                        nc.vector.tensor_copy(dn[:, lc * 512:(lc + 1) * 512],
                                              po[D:D + 1, :])

                    # Z row -> per-partition scalars: 16 PE transposes of
                    # [1,128] slices into one PSUM tile, then one reciprocal.
                    zps = smp.tile([128, 512], f32, tag="sm")
                    for lt in range(NT):
                        nc.tensor.transpose(zps[:, lt:lt + 1],
                                            dn[:, lt * 128:(lt + 1) * 128],
                                            ident[0:1, 0:1])
                    zrec = zrp.tile([128, NT], f32, tag="zr")
                    nc.vector.reciprocal(zrec[:], zps[:, 0:NT])

                    # --- D: logits -> exp -> normalize -> DMA --------------
                    for lt in range(NT):
                        qsl = qt[:, lt * 128:(lt + 1) * 128]
                        pa = ppool.tile([128, 1536], f32, tag="pp")
                        for c3 in range(3):
                            nc.tensor.matmul(
                                pa[:, c3 * 512:(c3 + 1) * 512], lhsT=qsl,
                                rhs=kt[:, c3 * 512:(c3 + 1) * 512],
                                start=True, stop=True)
                        pn = pnp.tile([128, S], f32, tag="pn")
                        nc.scalar.activation(pn[:, 0:1536], pa[:], Exp, scale=SCALE)
                        pb = ppool.tile([128, 1536], f32, tag="pp")
                        nc.tensor.matmul(pb[:, 0:512], lhsT=qsl,
                                         rhs=kt[:, 1536:2048], start=True, stop=True)
                        nc.scalar.activation(pn[:, 1536:2048], pb[:, 0:512], Exp,
                                             scale=SCALE)
                        nc.vector.tensor_scalar_mul(pn[:], pn[:], zrec[:, lt:lt + 1])
                        nc.sync.dma_start(scores_d[h, lt * 128:(lt + 1) * 128, :], pn[:])

                    # --- E: O = (O^T)^T * (1/Z) -> DMA ---------------------
                    osb = osp.tile([128, NT, D], f32, tag="os")
                    for lt in range(NT):
                        pt = smp.tile([128, 512], f32, tag="sm")
                        nc.tensor.transpose(pt[:, 0:D], ot[:, lt * 128:(lt + 1) * 128],
                                            ident[:D, :D])
                        nc.vector.tensor_scalar_mul(osb[:, lt, :], pt[:, 0:D],
                                                    zrec[:, lt:lt + 1])
                    nc.sync.dma_start(out_d[h].rearrange("(n p) d -> p n d", p=128),
                                      osb[:])
    _split_multi_waits(nc)
    return nc


def _get_prog(reps=1):
    if reps not in _PROG_CACHE:
        _PROG_CACHE[reps] = _build_program(reps)
    return _PROG_CACHE[reps]


def _shard(q, k, v):
    qf = np.ascontiguousarray(np.asarray(q, dtype=np.float32)).reshape(BH, L, D)
    kf = np.ascontiguousarray(np.asarray(k, dtype=np.float32)).reshape(BH, L, D)
    vf = np.ascontiguousarray(np.asarray(v, dtype=np.float32)).reshape(BH, L, D)
    return [
        {"q": np.ascontiguousarray(qf[c * NH:(c + 1) * NH]),
         "k": np.ascontiguousarray(kf[c * NH:(c + 1) * NH]),
         "v": np.ascontiguousarray(vf[c * NH:(c + 1) * NH])}
        for c in range(NCORES)
    ]


def _run(nc, in_maps, **kw):
    from concourse import bass_utils
    kw.setdefault("trace", False)
    return bass_utils.run_bass_kernel_spmd(
        nc, in_maps, core_ids=list(range(NCORES)), **kw)


def kernel(q, k, v):
    in_maps = _shard(q, k, v)
    res = _run(_get_prog(1), in_maps)
    out = np.concatenate([res.results[c]["out"] for c in range(NCORES)], axis=0)
    scores = np.concatenate([res.results[c]["scores"] for c in range(NCORES)], axis=0)
    return (out.reshape(B, H, L, D), scores.reshape(B, H, L, S))
